# revision 1
# baseline (speedup 1.0000x reference)
"""Additive attention (B=8, Q=K=512, H=Dv=64) on 8 TRN2 NeuronCores.

Math per batch b (reference):
    qf = queries @ Wq; kf = keys @ Wk
    scores[q,k] = sum_h wv[h] * tanh(qf[q,h] + kf[k,h])   (k >= valid_len masked)
    out = softmax_k(scores) @ values

Sharding: split-k, round-robin. Core c receives k-columns {c, c+8, ...} of
every batch (64 per batch), so the masked work (only k < valid_len matters)
is balanced across cores within +-1 column for any valid_lens. Each core
produces, per batch, a partial numerator  numT[d,q] = sum_{local k} p*v and
partial denominator den[q] = sum_{local k} p with p = exp(masked score);
the host sums partials over cores and divides. Softmax max-subtraction is
skipped (|scores| <= sum|wv| ~ 10, safe in f32; masked cols get exp(-1e9)=0).

Device kernel per core: 8 batch-slots. The broadcast add qf[q,:]+kf[k,:] is
done by the ScalarEngine's per-partition bias (one Tanh instruction per
k-pair over a [128=2k x 64h, 512q] tile); a sliding zero-padded wv matrix
contracts tanh features into score rows accumulated in PSUM; exp (with the
mask as per-partition bias) + the values matmul (with an appended ones
column producing the denominator row) finish the slot. Work is skipped at
runtime with Tile If blocks: per slot, groups of [16, 8, 8] k-pairs gated
on a per-core npairs input (PE PSUM base partitions only allow offsets
{0,32,64}, hence the scattered 80-row layout).

Projections (queries@Wq etc., 0.4% of FLOPs) and the k permutation are host
preprocessing; partial merge is host postprocessing.
"""
import numpy as np
from contextlib import ExitStack

B = 8
Q = 512
K = 512
H = 64
DV = 64
KLOC = 64              # local k columns per batch per core
NPAIR = KLOC // 2      # 32 k-pairs per slot
GROUPS = [(0, 16, 0), (16, 8, 32), (24, 8, 64)]   # (start_pair, n_pairs, psum row)
PROWS = 80             # psum/p rows used (scattered: k<48 -> r, k>=48 -> r+16)
NEG = -1e9


def _row_of(r):
    return r if r < 48 else r + 16


# ---------------------------------------------------------------------------
# BIR post-pass: the walrus build in this environment accepts only one
# sync-wait command per instruction; hoist extras onto same-engine NoOps.
def _split_waits(nc, k=1):
    import concourse.mybir as mybir
    n_new = 0
    for f in nc.m.functions:
        for bb in f.blocks:
            newlist = []
            for ins in bb.instructions:
                si = ins.sync_info
                if si is not None and si.on_wait and len(si.on_wait) > k:
                    waits = list(si.on_wait)
                    extra, keep = waits[:-k], waits[-k:]
                    for ci, w in enumerate(extra):
                        nop = mybir.InstNoOp(
                            name=f"{ins.name}_wsplit{ci}",
                            engine=ins.engine,
                            ins=[], outs=[],
                            sync_info=mybir.SyncInfo(on_wait=[w], on_update=[]),
                        )
                        newlist.append(nop)
                        n_new += 1
                    ins.sync_info = mybir.SyncInfo(
                        on_wait=list(keep), on_update=list(si.on_update))
                newlist.append(ins)
            bb.instructions[:] = newlist
    return n_new


def _build(nc, reps: int = 1):
    import concourse.bass as bass  # noqa: F401
    import concourse.mybir as mybir
    from concourse import tile

    F32 = mybir.dt.float32
    BF16 = mybir.dt.bfloat16
    I32 = mybir.dt.int32

    qf2 = nc.declare_dram_parameter("qf2", [128, B * Q], BF16, isOutput=False)
    kfp = nc.declare_dram_parameter("kfp", [128, B * NPAIR], F32, isOutput=False)
    vl = nc.declare_dram_parameter("vl", [B * PROWS, DV + 1], BF16, isOutput=False)
    wvall = nc.declare_dram_parameter("wvall", [128, 126 + 32], BF16, isOutput=False)
    maskcol = nc.declare_dram_parameter("maskcol", [PROWS, B], F32, isOutput=False)
    npairs = nc.declare_dram_parameter("npairs", [1, B], I32, isOutput=False)
    o65 = nc.declare_dram_parameter("o65", [DV + 1, B * Q], F32, isOutput=True)

    with tile.TileContext(nc) as tc:
        with (
            tc.tile_pool(name="cpool", bufs=1) as cpool,
            tc.tile_pool(name="fpool", bufs=4) as fpool,
            tc.tile_pool(name="ps_s", bufs=1, space="PSUM") as ps_s,
            tc.tile_pool(name="ps_o", bufs=3, space="PSUM") as ps_o,
        ):
            # prefetch the ACT tanh/exp table at t=0 (overlaps input DMAs)
            dummy = cpool.tile([1, 16], F32)
            nc.gpsimd.memset(dummy[:], 0.0)
            nc.scalar.activation(dummy[:], dummy[:],
                                 mybir.ActivationFunctionType.Tanh)
            np_sb = cpool.tile([1, B], I32)
            nc.sync.dma_start(np_sb[:], npairs[:])
            nvals = []
            for b in range(B):
                nvals.append(nc.values_load(
                    np_sb[:, b:b + 1], min_val=0, max_val=NPAIR,
                    skip_runtime_bounds_check=True))
            for rep in range(reps):
                qf2_sb = cpool.tile([128, B * Q], BF16)
                kfp_sb = cpool.tile([128, B * NPAIR], F32)
                vl_sb = cpool.tile([PROWS, B * (DV + 1)], BF16)
                wv_sb = cpool.tile([128, 126 + 32], BF16)
                mask_sb = cpool.tile([PROWS, B], F32)
                # order matters: earliest-needed first
                nc.sync.dma_start(wv_sb[:], wvall[:])
                nc.sync.dma_start(kfp_sb[:], kfp[:])
                nc.sync.dma_start(qf2_sb[:, 0:Q], qf2[:, 0:Q])
                nc.sync.dma_start(mask_sb[:], maskcol[:])
                for b in range(1, B):
                    nc.sync.dma_start(qf2_sb[:, b * Q:(b + 1) * Q],
                                      qf2[:, b * Q:(b + 1) * Q])
                nc.sync.dma_start(
                    vl_sb[:].rearrange("p (b d) -> p b d", b=B),
                    vl.rearrange("(b p) d -> p b d", p=PROWS),
                )

                o65_sb = cpool.tile([DV + 1, B * Q], F32)
                nc.vector.memset(o65_sb[:], 0.0)

                # static score banks, memset once so rows of never-executed
                # groups read as finite values (exp bias then zeroes them)
                ps_banks = []
                for i in range(2):
                    pb = ps_s.tile([PROWS, Q], mybir.dt.float32,
                                   tag=f"sc{i}", name=f"ps_bank{i}")
                    nc.vector.memset(pb[:], 0.0)
                    ps_banks.append(pb)

                for b in range(B):
                    with tc.If(nvals[b] > 0, name=f"slot{b}"):
                        ps_sc = ps_banks[b % 2]
                        with ExitStack() as stk:
                            for gi, (gstart, gn, goff) in enumerate(GROUPS):
                                if gi > 0:
                                    stk.enter_context(
                                        tc.If(nvals[b] > gstart, name=f"s{b}g{gi}"))
                                for u in range(gn):
                                    jj = gstart + u
                                    feat = fpool.tile([128, Q], BF16, tag="feat",
                                                      name=f"feat_{b}_{jj}")
                                    nc.scalar.activation(
                                        feat[:], qf2_sb[:, b * Q:(b + 1) * Q],
                                        mybir.ActivationFunctionType.Tanh,
                                        bias=kfp_sb[:, b * NPAIR + jj:
                                                    b * NPAIR + jj + 1])
                                    nc.tensor.matmul(
                                        ps_sc[goff: goff + 2 * gn, :],
                                        wv_sb[:, 126 - 2 * u: 126 - 2 * u + 2 * gn],
                                        feat[:],
                                        start=(u == 0), stop=(u == gn - 1))
                        p_sb = cpool.tile([PROWS, Q], BF16, tag="p", name=f"p_{b}")
                        nc.scalar.activation(
                            p_sb[:], ps_sc[:],
                            mybir.ActivationFunctionType.Exp,
                            bias=mask_sb[:, b:b + 1])
                        ps_oT = ps_o.tile([DV + 1, Q], mybir.dt.float32,
                                          tag="oT", name=f"ps_oT{b}")
                        nc.tensor.matmul(
                            ps_oT[:],
                            vl_sb[:, b * (DV + 1):(b + 1) * (DV + 1)],
                            p_sb[:], start=True, stop=True)
                        nc.vector.tensor_copy(
                            o65_sb[:, b * Q:(b + 1) * Q], ps_oT[:])
                    nc.sync.dma_start(o65[:, b * Q:(b + 1) * Q],
                                      o65_sb[:, b * Q:(b + 1) * Q])
    return nc


def host_inputs(queries, keys, values, valid_lens, Wq, Wk, wv):
    import ml_dtypes
    queries = np.asarray(queries, np.float32)
    keys = np.asarray(keys, np.float32)
    values = np.asarray(values, np.float32)
    Wq = np.asarray(Wq, np.float32)
    Wk = np.asarray(Wk, np.float32)
    wv = np.asarray(wv, np.float32)
    wvall = np.zeros((128, 126 + 32), np.float32)
    wvall[0:64, 126] = wv
    wvall[64:128, 127] = wv
    wvall = wvall.astype(ml_dtypes.bfloat16)
    qf = np.einsum("bqh,hi->biq", queries, Wq).astype(np.float32)   # [B, H, Q]
    kf = np.einsum("bkh,hi->bik", keys, Wk).astype(np.float32)      # [B, H, K]
    qf2 = np.concatenate([qf, qf], axis=1).reshape(B * 128, Q)
    qf2 = np.ascontiguousarray(
        qf2.reshape(B, 128, Q).transpose(1, 0, 2).reshape(128, B * Q)
    ).astype(ml_dtypes.bfloat16)
    rows = np.array([_row_of(r) for r in range(KLOC)])
    maps = []
    for c in range(8):
        kidx = np.arange(KLOC) * 8 + c
        kfp = np.zeros((128, B * NPAIR), np.float32)
        vla = np.zeros((B * PROWS, DV + 1), np.float32)
        nloc = np.zeros(B, np.int64)
        maskcol = np.full((PROWS, B), NEG, np.float32)
        for b in range(B):
            kl = kf[b][:, kidx]
            kfp[0:H, b * NPAIR:(b + 1) * NPAIR] = kl[:, 0::2]
            kfp[H:128, b * NPAIR:(b + 1) * NPAIR] = kl[:, 1::2]
            L = int(valid_lens[b])
            n = int(np.sum(kidx < L))
            nloc[b] = n
            vla[b * PROWS + rows, 0:DV] = values[b][kidx]
            vla[b * PROWS + rows, DV] = 1.0
            maskcol[rows[0:n], b] = 0.0
        npair = ((nloc + 1) // 2).astype(np.int32)
        maps.append({
            "qf2": qf2,
            "kfp": kfp,
            "vl": vla.astype(ml_dtypes.bfloat16),
            "wvall": wvall,
            "maskcol": maskcol,
            "npairs": npair[None, :],
        })
    return maps


def host_merge(results):
    acc = np.zeros((DV + 1, B * Q), np.float64)
    for c in range(8):
        acc += np.asarray(results[c]["o65"])
    acc = acc.reshape(DV + 1, B, Q)
    out = (acc[0:DV] / acc[DV][None, :, :]).transpose(1, 2, 0)
    return np.ascontiguousarray(out.astype(np.float32))


_RUNNER = None


def _get_runner():
    """Build + compile once per process; returns a callable(in_maps)->results."""
    global _RUNNER
    if _RUNNER is not None:
        return _RUNNER
    import jax
    from jax.sharding import Mesh, PartitionSpec
    from jax.experimental.shard_map import shard_map
    import concourse.bass as bass
    import concourse.mybir as mybir
    from concourse import bass2jax
    from concourse.bass2jax import _bass_exec_p, install_neuronx_cc_hook

    nc = bass.Bass()
    _build(nc)
    _split_waits(nc)

    install_neuronx_cc_hook()
    partition_name = nc.partition_id_tensor.name if nc.partition_id_tensor else None
    in_names, out_names, out_avals, zero_shapes = [], [], [], []
    for alloc in nc.m.functions[0].allocations:
        if not isinstance(alloc, mybir.MemoryLocationSet):
            continue
        name = alloc.memorylocations[0].name
        if alloc.kind == "ExternalInput":
            if name != partition_name:
                in_names.append(name)
        elif alloc.kind == "ExternalOutput":
            out_names.append(name)
            shape = tuple(alloc.tensor_shape)
            dtype = mybir.dt.np(alloc.dtype)
            out_avals.append(jax.core.ShapedArray(shape, dtype))
            zero_shapes.append((shape, dtype))
    n_params = len(in_names)
    n_outs = len(out_avals)
    in_names_all = in_names + out_names
    if partition_name is not None:
        in_names_all.append(partition_name)
    donate = tuple(range(n_params, n_params + n_outs))

    def _body(*args):
        operands = list(args)
        if partition_name is not None:
            operands.append(bass2jax.partition_id_tensor())
        outs = _bass_exec_p.bind(
            *operands,
            out_avals=tuple(out_avals),
            in_names=tuple(in_names_all),
            out_names=tuple(out_names),
            lowering_input_output_aliases=(),
            sim_require_finite=True,
            sim_require_nnan=True,
            nc=nc,
        )
        return tuple(outs)

    devices = jax.devices()[:8]
    mesh = Mesh(np.asarray(devices), ("core",))
    in_specs = (PartitionSpec("core"),) * (n_params + n_outs)
    out_specs = (PartitionSpec("core"),) * len(out_names)
    sharded = jax.jit(
        shard_map(_body, mesh=mesh, in_specs=in_specs, out_specs=out_specs,
                  check_rep=False),
        donate_argnums=donate, keep_unused=True,
    )

    def run(in_maps):
        per_core = [[np.asarray(m[name]) for name in in_names] for m in in_maps]
        concat_in = [
            np.concatenate([per_core[c][i] for c in range(8)], axis=0)
            for i in range(n_params)
        ]
        zeros = [np.zeros((8 * s[0],) + s[1:], d) for s, d in zero_shapes]
        out_arrs = sharded(*concat_in, *zeros)
        out_np = [np.asarray(a) for a in out_arrs]
        return [
            {name: out_np[i].reshape(8, *out_avals[i].shape)[c]
             for i, name in enumerate(out_names)}
            for c in range(8)
        ]

    _RUNNER = run
    return run


def kernel(queries, keys, values, valid_lens, Wq, Wk, wv):
    run = _get_runner()
    in_maps = host_inputs(queries, keys, values, valid_lens, Wq, Wk, wv)
    try:
        results = run(in_maps)
    except Exception:
        # transient NRT/axon failures have been observed; retry once
        results = run(in_maps)
    return host_merge(results)



# revision 2
# speedup vs baseline: 8.2834x; 8.2834x over previous
"""Additive attention (B=8, Q=K=512, H=Dv=64) on 8 TRN2 NeuronCores.

Math per batch b (reference):
    qf = queries @ Wq; kf = keys @ Wk
    scores[q,k] = sum_h wv[h] * tanh(qf[q,h] + kf[k,h])   (k >= valid_len masked)
    out = softmax_k(scores) @ values

Key idea: replace the pointwise tanh (134M ScalarEngine evaluations, ~93us)
with a low-rank bilinear expansion
    tanh(a+b) ~= sum_r phi_r(a) * psi_r(b),   r < R=12
obtained from the SVD of the kernel tanh(a+b) discretized on a grid with
sqrt-Gaussian row/column weighting (qf,kf entries are ~N(0,1)). Then
    scores[q,k] = sum_{r,h} Phi[q, r*64+h] * Psi[k, r*64+h]
is a plain matmul with contraction F = R*64 = 768 that the PE does in a few
microseconds. Rank-truncation rms error ~3e-4 per tanh; score error is
comparable (random-sign cancellation over wv) -> output rel err ~3e-3,
well inside the 2e-2 gate.

Sharding: data-parallel, one batch per core. Host computes qf/kf (0.4% of
FLOPs), evaluates the R basis functions per element (via table interp),
packs features into 128-row contraction chunks (2 ranks x 64 h); ranks 0-3
are shipped bf16, ranks 4-11 fp8(e4m3) with per-rank scale balancing
(error contribution scales with the decaying singular values). Device:
6 chunk-matmuls per 128-wide k-tile accumulate scores^T [k,512q] in PSUM
(fp8 pairs use DoubleRow perf mode, 2 contraction chunks per instruction),
masked exp via ACT per-partition bias, then a values matmul (ones column
appended -> denominator row). Host divides and transposes.
"""
import numpy as np
import ml_dtypes

B = 8
Q = 512
K = 512
H = 64
DV = 64

R = 12                 # SVD rank of tanh(a+b)
NBF = 2                # bf16 chunks (2 ranks each): ranks 0..3
NF8 = 4                # fp8 chunks: ranks 4..11 (DoubleRow pairs)
NCH = NBF + NF8
NTILE = K // 128       # 4 k-tiles
NEG = -1e9
F8MAX = 224.0          # ml_dtypes.float8_e4m3 max finite is 240
WARMUP_MM = 8          # PE p-state ramp fillers while input DMA streams

GRID_N, GRID_A, GRID_SIG, GRID_FLOOR = 1201, 6.5, 1.15, 0.02

_BASIS = None


def _basis():
    """SVD basis of tanh(a+b) on a weighted grid: x, phi[n,R], psi[n,R]."""
    global _BASIS
    if _BASIS is None:
        x = np.linspace(-GRID_A, GRID_A, GRID_N)
        Kg = np.tanh(x[:, None] + x[None, :])
        w = np.sqrt(np.exp(-x ** 2 / (2 * GRID_SIG ** 2))) + GRID_FLOOR
        U, S, Vt = np.linalg.svd((w[:, None] * Kg) * w[None, :])
        phi = (U[:, :R] * np.sqrt(S[:R])) / w[:, None]
        psi = (Vt[:R].T * np.sqrt(S[:R])) / w[:, None]
        _BASIS = (x, phi, psi)
    return _BASIS


# ---------------------------------------------------------------------------
# BIR post-pass: the walrus build in this environment accepts only one
# sync-wait command per instruction; hoist extras onto same-engine NoOps.
def _split_waits(nc, k=1):
    import concourse.mybir as mybir
    n_new = 0
    for f in nc.m.functions:
        for bb in f.blocks:
            newlist = []
            for ins in bb.instructions:
                si = ins.sync_info
                if si is not None and si.on_wait and len(si.on_wait) > k:
                    waits = list(si.on_wait)
                    extra, keep = waits[:-k], waits[-k:]
                    for ci, w in enumerate(extra):
                        nop = mybir.InstNoOp(
                            name=f"{ins.name}_wsplit{ci}",
                            engine=ins.engine,
                            ins=[], outs=[],
                            sync_info=mybir.SyncInfo(on_wait=[w], on_update=[]),
                        )
                        newlist.append(nop)
                        n_new += 1
                    ins.sync_info = mybir.SyncInfo(
                        on_wait=list(keep), on_update=list(si.on_update))
                newlist.append(ins)
            bb.instructions[:] = newlist
    return n_new


def _build(nc, reps: int = 1):
    import concourse.bass as bass  # noqa: F401
    import concourse.mybir as mybir
    from concourse import tile

    F32 = mybir.dt.float32
    BF16 = mybir.dt.bfloat16
    F8 = mybir.dt.float8e4
    DR = mybir.MatmulPerfMode.DoubleRow

    fb16 = nc.declare_dram_parameter("fb16", [NBF * 2 * 128, Q], BF16,
                                     isOutput=False)
    f8d = nc.declare_dram_parameter("f8d", [NF8 * 2 * 128, Q], F8,
                                    isOutput=False)
    vl1 = nc.declare_dram_parameter("vl1", [128, NTILE * (DV + 1)], BF16,
                                    isOutput=False)
    mbias = nc.declare_dram_parameter("mbias", [128, NTILE], F32,
                                      isOutput=False)
    o65 = nc.declare_dram_parameter("o65", [DV + 1, Q], F32, isOutput=True)

    with tile.TileContext(nc) as tc:  # noqa: F841
        with (
            tc.tile_pool(name="cpool", bufs=1) as cpool,
            tc.tile_pool(name="ppool", bufs=4) as ppool,
            tc.tile_pool(name="ps_s", bufs=1, space="PSUM") as ps_s,
            tc.tile_pool(name="ps_o", bufs=1, space="PSUM") as ps_o,
            tc.tile_pool(name="ps_w", bufs=1, space="PSUM") as ps_w,
        ):
            # PE p-state warmup + ACT exp-table prefetch during input DMA
            warm = cpool.tile([1, Q], BF16)
            nc.vector.memset(warm[:], 0.0)
            dummy = cpool.tile([1, 16], F32)
            nc.gpsimd.memset(dummy[:], 0.0)
            nc.scalar.activation(dummy[:], dummy[:],
                                 mybir.ActivationFunctionType.Exp)
            psw = ps_w.tile([16, Q], F32, tag="warm", name="psw")
            for i in range(WARMUP_MM):
                nc.tensor.matmul(psw[:], warm[:1, 0:16], warm[:],
                                 start=True, stop=True)

            for rep in range(reps):
                fb16_sb = cpool.tile([128, NBF * 2, Q], BF16,
                                     tag="fb16", name=f"fb16_{rep}")
                f8_sb = cpool.tile([128, NF8 // 2, 4, Q], F8,
                                   tag="f8", name=f"f8_{rep}")
                vl_sb = cpool.tile([128, NTILE, DV + 1], BF16,
                                   tag="vl", name=f"vl_{rep}")
                mb_sb = cpool.tile([128, NTILE], F32,
                                   tag="mb", name=f"mb_{rep}")
                nc.sync.dma_start(
                    fb16_sb[:], fb16.rearrange("(c p) n -> p c n", p=128))
                for g in range(NF8 // 2):
                    nc.sync.dma_start(
                        f8_sb[:, g],
                        f8d.rearrange("(g c p) n -> g p c n",
                                      g=NF8 // 2, p=128)[g])
                nc.sync.dma_start(
                    mb_sb[:], mbias[:])
                nc.sync.dma_start(
                    vl_sb[:], vl1.rearrange("p (t d) -> p t d", t=NTILE))

                sc = [ps_s.tile([128, Q], F32, tag=f"sc{t}",
                                name=f"sc{t}_{rep}") for t in range(NTILE)]
                po = ps_o.tile([DV + 1, Q], F32, tag="po", name=f"po_{rep}")

                # bf16 chunks, chunk-major so compute chases the DMA stream
                for c in range(NBF):
                    for t in range(NTILE):
                        nc.tensor.matmul(
                            sc[t][:],
                            fb16_sb[:, 2 * c, t * 128:(t + 1) * 128],
                            fb16_sb[:, 2 * c + 1, :],
                            start=(c == 0), stop=False)
                # fp8 DoubleRow groups; last group's stop interleaves with
                # exp + values-matmul per tile
                for g in range(NF8 // 2):
                    last = g == NF8 // 2 - 1
                    for t in range(NTILE):
                        nc.tensor.matmul(
                            sc[t][:],
                            f8_sb[:, g, 0:2, t * 128:(t + 1) * 128],
                            f8_sb[:, g, 2:4, :],
                            start=False, stop=last, perf_mode=DR)
                p_sb = []
                for t in range(NTILE):
                    p = ppool.tile([128, Q], BF16, tag="p", name=f"p{t}_{rep}")
                    nc.scalar.activation(
                        p[:], sc[t][:], mybir.ActivationFunctionType.Exp,
                        bias=mb_sb[:, t:t + 1])
                    p_sb.append(p)
                for t in range(NTILE):
                    nc.tensor.matmul(
                        po[:], vl_sb[:, t, :], p_sb[t][:],
                        start=(t == 0), stop=(t == NTILE - 1))
                o65_sb = cpool.tile([DV + 1, Q], F32,
                                    tag="o65", name=f"o65_{rep}")
                nc.vector.tensor_copy(o65_sb[:], po[:])
                nc.sync.dma_start(o65[:], o65_sb[:])
    return nc


def host_inputs(queries, keys, values, valid_lens, Wq, Wk, wv):
    x, phi, psi = _basis()
    queries = np.asarray(queries, np.float32)
    keys = np.asarray(keys, np.float32)
    values = np.asarray(values, np.float32)
    wv = np.asarray(wv, np.float32)
    qf = (queries @ np.asarray(Wq, np.float32)).astype(np.float32)  # [B,Q,H]
    kf = (keys @ np.asarray(Wk, np.float32)).astype(np.float32)     # [B,K,H]

    maps = []
    for b in range(B):
        Phi = np.stack([np.interp(qf[b], x, phi[:, r]) for r in range(R)],
                       1).astype(np.float32)              # [Q, R, H]
        Psi = np.stack([np.interp(kf[b], x, psi[:, r]) for r in range(R)],
                       1).astype(np.float32) * wv         # [K, R, H]
        mxq = np.abs(Phi).max(axis=(0, 2))
        mxk = np.abs(Psi).max(axis=(0, 2))
        alpha = np.sqrt(np.maximum(mxk, 1e-30) / np.maximum(mxq, 1e-30))
        Phi *= alpha[None, :, None]
        Psi /= alpha[None, :, None]
        # chunk c = ranks (2c, 2c+1): contraction row = 64*(r-2c) + h
        PhiT = Phi.reshape(Q, R * H).T      # [768, Q]
        PsiT = Psi.reshape(K, R * H).T      # [768, K]
        blocks16 = []
        for c in range(NBF):
            blocks16 += [PsiT[c * 128:(c + 1) * 128],
                         PhiT[c * 128:(c + 1) * 128]]
        fb = np.concatenate(blocks16, 0).astype(ml_dtypes.bfloat16)
        blocks8 = []
        for g in range(NF8 // 2):
            c0 = NBF + 2 * g
            blocks8 += [PsiT[c0 * 128:(c0 + 1) * 128],
                        PsiT[(c0 + 1) * 128:(c0 + 2) * 128],
                        PhiT[c0 * 128:(c0 + 1) * 128],
                        PhiT[(c0 + 1) * 128:(c0 + 2) * 128]]
        f8 = np.clip(np.concatenate(blocks8, 0), -F8MAX, F8MAX)
        f8 = f8.astype(ml_dtypes.float8_e4m3)

        vla = np.zeros((128, NTILE * (DV + 1)), np.float32)
        for t in range(NTILE):
            vla[:, t * (DV + 1):t * (DV + 1) + DV] = \
                values[b][t * 128:(t + 1) * 128]
            vla[:, t * (DV + 1) + DV] = 1.0
        L = int(valid_lens[b])
        kidx = np.arange(K).reshape(NTILE, 128).T    # [128, NTILE]
        mb = np.where(kidx < L, 0.0, NEG).astype(np.float32)
        maps.append({
            "fb16": fb,
            "f8d": f8,
            "vl1": vla.astype(ml_dtypes.bfloat16),
            "mbias": mb,
        })
    return maps


def host_merge(results):
    out = np.empty((B, Q, DV), np.float32)
    for b in range(B):
        o = np.asarray(results[b]["o65"], np.float32)   # [65, Q]
        out[b] = (o[0:DV] / o[DV][None, :]).T
    return np.ascontiguousarray(out)


_RUNNER = None


def _get_runner():
    """Build + compile once per process; returns a callable(in_maps)->results."""
    global _RUNNER
    if _RUNNER is not None:
        return _RUNNER
    import jax
    from jax.sharding import Mesh, PartitionSpec
    from jax.experimental.shard_map import shard_map
    import concourse.bass as bass
    import concourse.mybir as mybir
    from concourse import bass2jax
    from concourse.bass2jax import _bass_exec_p, install_neuronx_cc_hook

    nc = bass.Bass()
    _build(nc)
    _split_waits(nc)

    install_neuronx_cc_hook()
    partition_name = nc.partition_id_tensor.name if nc.partition_id_tensor else None
    in_names, out_names, out_avals, zero_shapes = [], [], [], []
    for alloc in nc.m.functions[0].allocations:
        if not isinstance(alloc, mybir.MemoryLocationSet):
            continue
        name = alloc.memorylocations[0].name
        if alloc.kind == "ExternalInput":
            if name != partition_name:
                in_names.append(name)
        elif alloc.kind == "ExternalOutput":
            out_names.append(name)
            shape = tuple(alloc.tensor_shape)
            dtype = mybir.dt.np(alloc.dtype)
            out_avals.append(jax.core.ShapedArray(shape, dtype))
            zero_shapes.append((shape, dtype))
    n_params = len(in_names)
    n_outs = len(out_avals)
    in_names_all = in_names + out_names
    if partition_name is not None:
        in_names_all.append(partition_name)
    donate = tuple(range(n_params, n_params + n_outs))

    def _body(*args):
        operands = list(args)
        if partition_name is not None:
            operands.append(bass2jax.partition_id_tensor())
        outs = _bass_exec_p.bind(
            *operands,
            out_avals=tuple(out_avals),
            in_names=tuple(in_names_all),
            out_names=tuple(out_names),
            lowering_input_output_aliases=(),
            sim_require_finite=True,
            sim_require_nnan=True,
            nc=nc,
        )
        return tuple(outs)

    devices = jax.devices()[:8]
    mesh = Mesh(np.asarray(devices), ("core",))
    in_specs = (PartitionSpec("core"),) * (n_params + n_outs)
    out_specs = (PartitionSpec("core"),) * len(out_names)
    sharded = jax.jit(
        shard_map(_body, mesh=mesh, in_specs=in_specs, out_specs=out_specs,
                  check_rep=False),
        donate_argnums=donate, keep_unused=True,
    )

    def run(in_maps):
        per_core = [[np.asarray(m[name]) for name in in_names] for m in in_maps]
        concat_in = [
            np.concatenate([per_core[c][i] for c in range(8)], axis=0)
            for i in range(n_params)
        ]
        zeros = [np.zeros((8 * s[0],) + s[1:], d) for s, d in zero_shapes]
        out_arrs = sharded(*concat_in, *zeros)
        out_np = [np.asarray(a) for a in out_arrs]
        return [
            {name: out_np[i].reshape(8, *out_avals[i].shape)[c]
             for i, name in enumerate(out_names)}
            for c in range(8)
        ]

    _RUNNER = run
    return run


def kernel(queries, keys, values, valid_lens, Wq, Wk, wv):
    run = _get_runner()
    in_maps = host_inputs(queries, keys, values, valid_lens, Wq, Wk, wv)
    try:
        results = run(in_maps)
    except Exception:
        # transient NRT/axon failures have been observed; retry once
        results = run(in_maps)
    return host_merge(results)


# revision 3
# speedup vs baseline: 8.9881x; 1.0851x over previous
"""Additive attention (B=8, Q=K=512, H=Dv=64) on 8 TRN2 NeuronCores.

Math per batch b (reference):
    qf = queries @ Wq; kf = keys @ Wk
    scores[q,k] = sum_h wv[h] * tanh(qf[q,h] + kf[k,h])   (k >= valid_len masked)
    out = softmax_k(scores) @ values

Key idea: replace the pointwise tanh (134M ScalarEngine evaluations, ~93us)
with a low-rank bilinear expansion
    tanh(a+b) ~= sum_r phi_r(a) * psi_r(b),   r < R=10
obtained from the SVD of the kernel tanh(a+b) discretized on a grid with
sqrt-Gaussian row/column weighting (qf,kf entries are ~N(0,1)). Then
    scores[q,k] = sum_{r,h} Phi[q, r*64+h] * Psi[k, r*64+h]
is a plain matmul with contraction F = R*64 = 640 done on the PE.

Sharding: data-parallel, one batch per core. Host computes qf/kf (0.4% of
FLOPs), evaluates the R basis functions per element (table interp), packs
features into 128-row contraction chunks (2 ranks x 64 h); ranks 0-1 ship
bf16, ranks 2-9 fp8(e4m3) with per-rank scale balancing (rank errors scale
with the decaying singular values). The key-side softmax mask is FOLDED
into the features: the (rank 1, argmin|wv|) slot is repurposed as
Phi=1 / Psi = 0 or -60000, so masked columns get score ~ -6e4 and exp -> 0
with no per-partition bias needed (the stolen slot's term is ~|wv|_min,
negligible). Device per core: 16 chunk-matmuls accumulate scores^T
[4 k-tiles x 128, 512q] into one 4-bank PSUM tile (fp8 chunk pairs use
DoubleRow, 2 contraction chunks per instruction), ONE merged exp over all
4 banks -> p bf16, 4 values-matmuls (ones column -> denominator row),
copy + DMA out. Host divides and transposes. Early dummy matmuls keep the
PE p-state ramp warm while inputs stream.
"""
import numpy as np
import ml_dtypes

B = 8
Q = 512
K = 512
H = 64
DV = 64

R = 10                 # SVD rank of tanh(a+b)
NBF = 1                # bf16 chunks (2 ranks each): ranks 0..1
NF8 = 4                # fp8 chunks: ranks 2..9 (DoubleRow pairs)
NCH = NBF + NF8
NTILE = K // 128       # 4 k-tiles
MASKBIG = -60000.0
F8MAX = 224.0          # ml_dtypes.float8_e4m3 max finite is 240
WARMUP_MM = 7          # PE p-state ramp fillers while input DMA streams

GRID_N, GRID_A, GRID_SIG, GRID_FLOOR = 1201, 6.5, 1.15, 0.02

_BASIS = None


def _basis():
    """SVD basis of tanh(a+b) on a weighted grid: x, phi[n,R], psi[n,R]."""
    global _BASIS
    if _BASIS is None:
        x = np.linspace(-GRID_A, GRID_A, GRID_N)
        Kg = np.tanh(x[:, None] + x[None, :])
        w = np.sqrt(np.exp(-x ** 2 / (2 * GRID_SIG ** 2))) + GRID_FLOOR
        U, S, Vt = np.linalg.svd((w[:, None] * Kg) * w[None, :])
        phi = (U[:, :R] * np.sqrt(S[:R])) / w[:, None]
        psi = (Vt[:R].T * np.sqrt(S[:R])) / w[:, None]
        _BASIS = (x, phi, psi)
    return _BASIS


# ---------------------------------------------------------------------------
# BIR post-pass: the walrus build in this environment accepts only one
# sync-wait command per instruction; hoist extras onto same-engine NoOps.
def _split_waits(nc, k=1):
    import concourse.mybir as mybir
    n_new = 0
    for f in nc.m.functions:
        for bb in f.blocks:
            newlist = []
            for ins in bb.instructions:
                si = ins.sync_info
                if si is not None and si.on_wait and len(si.on_wait) > k:
                    waits = list(si.on_wait)
                    extra, keep = waits[:-k], waits[-k:]
                    for ci, w in enumerate(extra):
                        nop = mybir.InstNoOp(
                            name=f"{ins.name}_wsplit{ci}",
                            engine=ins.engine,
                            ins=[], outs=[],
                            sync_info=mybir.SyncInfo(on_wait=[w], on_update=[]),
                        )
                        newlist.append(nop)
                        n_new += 1
                    ins.sync_info = mybir.SyncInfo(
                        on_wait=list(keep), on_update=list(si.on_update))
                newlist.append(ins)
            bb.instructions[:] = newlist
    return n_new


def _build(nc, reps: int = 1):
    import concourse.bass as bass  # noqa: F401
    import concourse.mybir as mybir
    from concourse import tile

    F32 = mybir.dt.float32
    BF16 = mybir.dt.bfloat16
    F8 = mybir.dt.float8e4
    DR = mybir.MatmulPerfMode.DoubleRow

    fb16 = nc.declare_dram_parameter("fb16", [NBF * 2 * 128, Q], BF16,
                                     isOutput=False)
    f8d = nc.declare_dram_parameter("f8d", [NF8 * 2 * 128, Q], F8,
                                    isOutput=False)
    vl1 = nc.declare_dram_parameter("vl1", [128, NTILE * (DV + 1)], BF16,
                                    isOutput=False)
    o65 = nc.declare_dram_parameter("o65", [DV + 1, Q], F32, isOutput=True)

    with tile.TileContext(nc) as tc:  # noqa: F841
        with (
            tc.tile_pool(name="cpool", bufs=1) as cpool,
            tc.tile_pool(name="ppool", bufs=2) as ppool,
            tc.tile_pool(name="ps_s", bufs=1, space="PSUM") as ps_s,
            tc.tile_pool(name="ps_o", bufs=1, space="PSUM") as ps_o,
            tc.tile_pool(name="ps_w", bufs=1, space="PSUM") as ps_w,
        ):
            # PE p-state warmup + ACT exp-table prefetch during input DMA
            warm = cpool.tile([1, Q], BF16)
            nc.gpsimd.memset(warm[:], 0.0)
            dummy = cpool.tile([1, 16], F32)
            nc.gpsimd.memset(dummy[:], 0.0)
            nc.scalar.activation(dummy[:], dummy[:],
                                 mybir.ActivationFunctionType.Exp)
            psw = ps_w.tile([16, Q], F32, tag="warm", name="psw")
            for i in range(WARMUP_MM):
                nc.tensor.matmul(psw[:], warm[:1, 0:16], warm[:],
                                 start=True, stop=True)

            for rep in range(reps):
                fb16_sb = cpool.tile([128, NBF * 2, Q], BF16,
                                     tag="fb16", name=f"fb16_{rep}")
                f8_sb = cpool.tile([128, NF8 // 2, 4, Q], F8,
                                   tag="f8", name=f"f8_{rep}")
                vl_sb = cpool.tile([128, NTILE, DV + 1], BF16,
                                   tag="vl", name=f"vl_{rep}")
                nc.sync.dma_start(
                    fb16_sb[:], fb16.rearrange("(c p) n -> p c n", p=128))
                for g in range(NF8 // 2):
                    nc.sync.dma_start(
                        f8_sb[:, g],
                        f8d.rearrange("(g c p) n -> g p c n",
                                      g=NF8 // 2, p=128)[g])
                nc.sync.dma_start(
                    vl_sb[:], vl1.rearrange("p (t d) -> p t d", t=NTILE))

                sc = ps_s.tile([128, NTILE, Q], F32, tag="sc",
                               name=f"sc_{rep}")
                po = ps_o.tile([DV + 1, Q], F32, tag="po", name=f"po_{rep}")

                # bf16 chunks, chunk-major so compute chases the DMA stream
                for c in range(NBF):
                    for t in range(NTILE):
                        nc.tensor.matmul(
                            sc[:, t, :],
                            fb16_sb[:, 2 * c, t * 128:(t + 1) * 128],
                            fb16_sb[:, 2 * c + 1, :],
                            start=(c == 0), stop=False)
                # fp8 DoubleRow groups (2 contraction chunks per matmul)
                for g in range(NF8 // 2):
                    last = g == NF8 // 2 - 1
                    for t in range(NTILE):
                        nc.tensor.matmul(
                            sc[:, t, :],
                            f8_sb[:, g, 0:2, t * 128:(t + 1) * 128],
                            f8_sb[:, g, 2:4, :],
                            start=False, stop=last, perf_mode=DR)
                # one merged exp over all 4 k-tiles (mask folded in scores)
                p_sb = ppool.tile([128, NTILE, Q], BF16, tag="p",
                                  name=f"p_{rep}")
                nc.scalar.activation(
                    p_sb[:], sc[:], mybir.ActivationFunctionType.Exp)
                for t in range(NTILE):
                    nc.tensor.matmul(
                        po[:], vl_sb[:, t, :], p_sb[:, t, :],
                        start=(t == 0), stop=(t == NTILE - 1))
                o65_sb = cpool.tile([DV + 1, Q], F32,
                                    tag="o65", name=f"o65_{rep}")
                nc.vector.tensor_copy(o65_sb[:], po[:])
                nc.sync.dma_start(o65[:], o65_sb[:])
    return nc


def host_inputs(queries, keys, values, valid_lens, Wq, Wk, wv):
    x, phi, psi = _basis()
    queries = np.asarray(queries, np.float32)
    keys = np.asarray(keys, np.float32)
    values = np.asarray(values, np.float32)
    wv = np.asarray(wv, np.float32)
    qf = (queries @ np.asarray(Wq, np.float32)).astype(np.float32)  # [B,Q,H]
    kf = (keys @ np.asarray(Wk, np.float32)).astype(np.float32)     # [B,K,H]
    hmin = int(np.argmin(np.abs(wv)))

    maps = []
    for b in range(B):
        Phi = np.stack([np.interp(qf[b], x, phi[:, r]) for r in range(R)],
                       1).astype(np.float32)              # [Q, R, H]
        Psi = np.stack([np.interp(kf[b], x, psi[:, r]) for r in range(R)],
                       1).astype(np.float32) * wv         # [K, R, H]
        mxq = np.abs(Phi).max(axis=(0, 2))
        mxk = np.abs(Psi).max(axis=(0, 2))
        alpha = np.sqrt(np.maximum(mxk, 1e-30) / np.maximum(mxq, 1e-30))
        Phi *= alpha[None, :, None]
        Psi /= alpha[None, :, None]
        # fold the key mask into the (rank 2*NBF-1, argmin|wv|) slot
        L = int(valid_lens[b])
        rm = 2 * NBF - 1
        Phi[:, rm, hmin] = 1.0
        Psi[:, rm, hmin] = np.where(np.arange(K) < L, 0.0, MASKBIG)
        # chunk c = ranks (2c, 2c+1): contraction row = 64*(r-2c) + h
        PhiT = Phi.reshape(Q, R * H).T      # [F, Q]
        PsiT = Psi.reshape(K, R * H).T      # [F, K]
        blocks16 = []
        for c in range(NBF):
            blocks16 += [PsiT[c * 128:(c + 1) * 128],
                         PhiT[c * 128:(c + 1) * 128]]
        fb = np.concatenate(blocks16, 0).astype(ml_dtypes.bfloat16)
        blocks8 = []
        for g in range(NF8 // 2):
            c0 = NBF + 2 * g
            blocks8 += [PsiT[c0 * 128:(c0 + 1) * 128],
                        PsiT[(c0 + 1) * 128:(c0 + 2) * 128],
                        PhiT[c0 * 128:(c0 + 1) * 128],
                        PhiT[(c0 + 1) * 128:(c0 + 2) * 128]]
        f8 = np.clip(np.concatenate(blocks8, 0), -F8MAX, F8MAX)
        f8 = f8.astype(ml_dtypes.float8_e4m3)

        vla = np.zeros((128, NTILE * (DV + 1)), np.float32)
        for t in range(NTILE):
            vla[:, t * (DV + 1):t * (DV + 1) + DV] = \
                values[b][t * 128:(t + 1) * 128]
            vla[:, t * (DV + 1) + DV] = 1.0
        maps.append({
            "fb16": fb,
            "f8d": f8,
            "vl1": vla.astype(ml_dtypes.bfloat16),
        })
    return maps


def host_merge(results):
    out = np.empty((B, Q, DV), np.float32)
    for b in range(B):
        o = np.asarray(results[b]["o65"], np.float32)   # [65, Q]
        out[b] = (o[0:DV] / o[DV][None, :]).T
    return np.ascontiguousarray(out)


_RUNNER = None


def _get_runner():
    """Build + compile once per process; returns a callable(in_maps)->results."""
    global _RUNNER
    if _RUNNER is not None:
        return _RUNNER
    import jax
    from jax.sharding import Mesh, PartitionSpec
    from jax.experimental.shard_map import shard_map
    import concourse.bass as bass
    import concourse.mybir as mybir
    from concourse import bass2jax
    from concourse.bass2jax import _bass_exec_p, install_neuronx_cc_hook

    nc = bass.Bass()
    _build(nc)
    _split_waits(nc)

    install_neuronx_cc_hook()
    partition_name = nc.partition_id_tensor.name if nc.partition_id_tensor else None
    in_names, out_names, out_avals, zero_shapes = [], [], [], []
    for alloc in nc.m.functions[0].allocations:
        if not isinstance(alloc, mybir.MemoryLocationSet):
            continue
        name = alloc.memorylocations[0].name
        if alloc.kind == "ExternalInput":
            if name != partition_name:
                in_names.append(name)
        elif alloc.kind == "ExternalOutput":
            out_names.append(name)
            shape = tuple(alloc.tensor_shape)
            dtype = mybir.dt.np(alloc.dtype)
            out_avals.append(jax.core.ShapedArray(shape, dtype))
            zero_shapes.append((shape, dtype))
    n_params = len(in_names)
    n_outs = len(out_avals)
    in_names_all = in_names + out_names
    if partition_name is not None:
        in_names_all.append(partition_name)
    donate = tuple(range(n_params, n_params + n_outs))

    def _body(*args):
        operands = list(args)
        if partition_name is not None:
            operands.append(bass2jax.partition_id_tensor())
        outs = _bass_exec_p.bind(
            *operands,
            out_avals=tuple(out_avals),
            in_names=tuple(in_names_all),
            out_names=tuple(out_names),
            lowering_input_output_aliases=(),
            sim_require_finite=True,
            sim_require_nnan=True,
            nc=nc,
        )
        return tuple(outs)

    devices = jax.devices()[:8]
    mesh = Mesh(np.asarray(devices), ("core",))
    in_specs = (PartitionSpec("core"),) * (n_params + n_outs)
    out_specs = (PartitionSpec("core"),) * len(out_names)
    sharded = jax.jit(
        shard_map(_body, mesh=mesh, in_specs=in_specs, out_specs=out_specs,
                  check_rep=False),
        donate_argnums=donate, keep_unused=True,
    )

    def run(in_maps):
        per_core = [[np.asarray(m[name]) for name in in_names] for m in in_maps]
        concat_in = [
            np.concatenate([per_core[c][i] for c in range(8)], axis=0)
            for i in range(n_params)
        ]
        zeros = [np.zeros((8 * s[0],) + s[1:], d) for s, d in zero_shapes]
        out_arrs = sharded(*concat_in, *zeros)
        out_np = [np.asarray(a) for a in out_arrs]
        return [
            {name: out_np[i].reshape(8, *out_avals[i].shape)[c]
             for i, name in enumerate(out_names)}
            for c in range(8)
        ]

    _RUNNER = run
    return run


def kernel(queries, keys, values, valid_lens, Wq, Wk, wv):
    run = _get_runner()
    in_maps = host_inputs(queries, keys, values, valid_lens, Wq, Wk, wv)
    try:
        results = run(in_maps)
    except Exception:
        # transient NRT/axon failures have been observed; retry once
        results = run(in_maps)
    return host_merge(results)
